# revision 11
# baseline (speedup 1.0000x reference)
"""KernelConv2D (per-pixel dynamic 5x5 depthwise conv) on 8 TRN2 NeuronCores.

Problem: out[b,c,h,w] = sum_{i,j} x_edgepad[b,c,h+i,w+j] * K[b,c,i,j,h,w]
with input [4,32,128,128] f32 and kernel [4,800,128,128] f32 (800 = 32*25).

Sharding: every (b,c) plane is independent, so flatten to 128 planes and put
the plane index on the SBUF partition axis. Each core takes 16 output ROWS of
all 128 planes (row-sharding). With (h, w) both living in the free dimension,
both conv shifts are constant free-dim offsets -> the 5x5 taps of the input
window are expressed as a single overlapping access pattern, no halo exchange
or partition-shifted copies on device. Host pre-pads the input with edge
replication and slices per-core row bands (incl. 2-row halo).

Per core HBM traffic: K 26.2MB + X 1.35MB + out 1.05MB ~= 28.6MB -> ~80us at
~358 GB/s/core: the memory roofline for this problem. Compute is split so DVE
(products + 9-segment reduce) and GpSimd (16-segment add tree) both stay at or
under the DMA time.
"""

import sys

import numpy as np

sys.path.insert(0, "/opt/trn_rl_repo")

import concourse.bacc as bacc
import concourse.bass as bass
import concourse.tile as tile
from concourse import mybir
from concourse.ap import AP
from concourse.bass_utils import run_bass_kernel_spmd

N_CORES = 8
B, C, H, W, KS = 4, 32, 128, 128, 5
NPLANES = B * C          # 128 -> partition axis
NTAPS = KS * KS          # 25
ROWS_PER_CORE = H // N_CORES   # 16
ROWS_PER_CHUNK = 2
NCHUNK = ROWS_PER_CORE // ROWS_PER_CHUNK   # 8
FDW = ROWS_PER_CHUNK * W                   # 256 output elems per chunk-partition
XW = W + KS - 1                            # 132 padded row width
XROWS = ROWS_PER_CORE + KS - 1             # 20 rows incl halo
F32 = mybir.dt.float32

# Reduction split: DVE sums taps [0, 9) with contiguous pairwise tree adds;
# GpSimd sums taps [9, 25) with a power-of-2 tree and does the final combine
# (so the Vector FIFO never waits on GpSimd -> no head-of-line blocking).
DVE_SEGS = 9

_compiled = None


def _build_program():
    nc = bacc.Bacc(
        "TRN2",
        target_bir_lowering=False,
        debug=False,
        enable_asserts=False,
        num_devices=N_CORES,
    )
    # Host pre-arranges k as [plane][chunk][tap][h2][w] so each chunk load is
    # one contiguous per-partition run (few DMA descriptors, near line rate).
    xd = nc.declare_dram_parameter("x", [NPLANES, XROWS * XW], F32, isOutput=False)
    kd = nc.declare_dram_parameter(
        "k", [NPLANES, NCHUNK * NTAPS * FDW], F32, isOutput=False
    )
    od = nc.declare_dram_parameter("o", [NPLANES, NCHUNK * FDW], F32, isOutput=True)

    with tile.TileContext(nc) as tc:
        with (
            tc.tile_pool(name="xpool", bufs=1) as xpool,
            tc.tile_pool(name="kpool", bufs=2) as kpool,
            tc.tile_pool(name="ppool", bufs=2) as ppool,
            tc.tile_pool(name="gpool", bufs=2) as gpool,
            tc.tile_pool(name="dpool", bufs=2) as dpool,
            tc.tile_pool(name="opool", bufs=3) as opool,
        ):
            # Whole padded input band for this core, resident for the kernel.
            xt = xpool.tile([NPLANES, XROWS * XW], F32)
            nc.sync.dma_start(out=xt[:], in_=xd.ap())
            xt_ap = xt[:]
            xt_pdim = xt_ap.ap[0]  # (partition step, 128)

            for ch in range(NCHUNK):
                h0 = ch * ROWS_PER_CHUNK
                kt = kpool.tile([NPLANES, NTAPS * FDW], F32, tag="kt")
                pt = ppool.tile([NPLANES, NTAPS * FDW], F32, tag="pt")
                # Products: per vertical tap i, load its 5-tap K row group and
                # multiply against an overlapping strided window of the X band
                # (sub-loads let products start before the whole chunk lands).
                for i in range(KS):
                    seg = KS * FDW
                    nc.sync.dma_start(
                        out=kt[:, i * seg : (i + 1) * seg],
                        in_=kd.ap()[
                            :,
                            ch * NTAPS * FDW + i * seg : ch * NTAPS * FDW
                            + (i + 1) * seg,
                        ],
                    )
                    k_view = kt[:, i * seg : (i + 1) * seg].rearrange(
                        "p (j h w) -> p j h w", j=KS, h=ROWS_PER_CHUNK, w=W
                    )
                    p_view = pt[:, i * seg : (i + 1) * seg].rearrange(
                        "p (j h w) -> p j h w", j=KS, h=ROWS_PER_CHUNK, w=W
                    )
                    x_view = AP(
                        xt_ap.tensor,
                        xt_ap.offset + (h0 + i) * XW,
                        [xt_pdim, (1, KS), (XW, ROWS_PER_CHUNK), (1, W)],
                    )
                    nc.vector.tensor_mul(p_view, k_view, x_view)

                def ps(a, b):  # pt tap-segment range [a, b)
                    return pt[:, a * FDW : b * FDW]

                # DVE: contiguous pairwise tree over taps [0, 9).
                dt = dpool.tile([NPLANES, 8 * FDW], F32, tag="dt")

                def ds(a, b):
                    return dt[:, a * FDW : b * FDW]

                nc.vector.tensor_add(ds(0, 4), ps(0, 4), ps(4, 8))
                nc.vector.tensor_add(ds(4, 6), ds(0, 2), ds(2, 4))
                nc.vector.tensor_add(ds(6, 7), ds(4, 5), ds(5, 6))
                nc.vector.tensor_add(ds(7, 8), ds(6, 7), ps(8, 9))

                # GpSimd: power-of-2 tree over taps [9, 25) + final combine.
                gt = gpool.tile([NPLANES, 15 * FDW], F32, tag="gt")

                def gs(a, b):
                    return gt[:, a * FDW : b * FDW]

                nc.gpsimd.tensor_add(gs(0, 8), ps(9, 17), ps(17, 25))
                nc.gpsimd.tensor_add(gs(8, 12), gs(0, 4), gs(4, 8))
                nc.gpsimd.tensor_add(gs(12, 14), gs(8, 10), gs(10, 12))
                nc.gpsimd.tensor_add(gs(14, 15), gs(12, 13), gs(13, 14))
                ot = opool.tile([NPLANES, FDW], F32, tag="ot")
                nc.gpsimd.tensor_add(ot[:], gs(14, 15), ds(7, 8))
                nc.sync.dma_start(
                    out=od.ap()[:, ch * FDW : (ch + 1) * FDW], in_=ot[:]
                )

    nc.compile()
    return nc


def _get_program():
    global _compiled
    if _compiled is None:
        _compiled = _build_program()
    return _compiled


def _shard_inputs(input: np.ndarray, kernel: np.ndarray):
    x = np.ascontiguousarray(input, dtype=np.float32).reshape(NPLANES, H, W)
    xp = np.pad(x, ((0, 0), (2, 2), (2, 2)), mode="edge")  # [128, 132, 132]
    k = np.ascontiguousarray(kernel, dtype=np.float32).reshape(
        NPLANES, NTAPS, H, W
    )
    in_maps = []
    for c in range(N_CORES):
        r0 = c * ROWS_PER_CORE
        # [plane][tap][16 rows][w] -> [plane][chunk][tap][2 rows][w], flattened
        # per plane so each chunk is one contiguous run.
        kc = k[:, :, r0 : r0 + ROWS_PER_CORE, :].reshape(
            NPLANES, NTAPS, NCHUNK, ROWS_PER_CHUNK, W
        )
        kc = np.ascontiguousarray(kc.transpose(0, 2, 1, 3, 4)).reshape(
            NPLANES, NCHUNK * NTAPS * FDW
        )
        in_maps.append(
            {
                "x": np.ascontiguousarray(
                    xp[:, r0 : r0 + XROWS, :]
                ).reshape(NPLANES, XROWS * XW),
                "k": kc,
            }
        )
    return in_maps


last_results = None  # BassKernelResults of the most recent run (for profiling)


def kernel(input: np.ndarray, kernel: np.ndarray, _trace: bool = False):
    global last_results
    nc = _get_program()
    in_maps = _shard_inputs(input, kernel)
    res = run_bass_kernel_spmd(nc, in_maps, list(range(N_CORES)), trace=_trace)
    last_results = res
    out = np.empty((NPLANES, H, W), dtype=np.float32)
    for c in range(N_CORES):
        out[:, c * ROWS_PER_CORE : (c + 1) * ROWS_PER_CORE, :] = res.results[c][
            "o"
        ].reshape(NPLANES, ROWS_PER_CORE, W)
    return out.reshape(B, C, H, W)


if __name__ == "__main__":
    rng = np.random.default_rng(0)
    inp = rng.standard_normal((B, C, H, W), dtype=np.float32)
    kern = rng.standard_normal((B, C * NTAPS, H, W), dtype=np.float32)
    out = kernel(inp, kern)
    print("ran ok", out.shape, out.dtype)


# revision 16
# speedup vs baseline: 1.6005x; 1.6005x over previous
"""KernelConv2D (per-pixel dynamic 5x5 depthwise conv) on 8 TRN2 NeuronCores.

Problem: out[b,c,h,w] = sum_{i,j} x_edgepad[b,c,h+i,w+j] * K[b,c,i,j,h,w]
with input [4,32,128,128] f32 and kernel [4,800,128,128] f32 (800 = 32*25).

Sharding: every (b,c) plane is independent, so flatten to 128 planes and put
the plane index on the SBUF partition axis. Each core takes 16 output ROWS of
all 128 planes (row-sharding). With (h, w) both living in the free dimension,
both conv shifts are constant free-dim offsets -> the 5x5 taps of the input
window are expressed as a single overlapping access pattern, no halo exchange
or partition-shifted copies on device. Host pre-pads the input with edge
replication and slices per-core row bands (incl. 2-row halo).

Per core HBM traffic: K 26.2MB + X 1.35MB + out 1.05MB ~= 28.6MB -> ~80us at
~358 GB/s/core: the memory roofline for this problem. Compute is split so DVE
(products + 9-segment reduce) and GpSimd (16-segment add tree) both stay at or
under the DMA time.
"""

import sys

import numpy as np

sys.path.insert(0, "/opt/trn_rl_repo")

import concourse.bacc as bacc
import concourse.bass as bass
import concourse.tile as tile
from concourse import mybir
from concourse.ap import AP
from concourse.bass_utils import run_bass_kernel_spmd

N_CORES = 8
B, C, H, W, KS = 4, 32, 128, 128, 5
NPLANES = B * C          # 128 -> partition axis
NTAPS = KS * KS          # 25
ROWS_PER_CORE = H // N_CORES   # 16
ROWS_PER_CHUNK = 2
NCHUNK = ROWS_PER_CORE // ROWS_PER_CHUNK   # 8
FDW = ROWS_PER_CHUNK * W                   # 256 output elems per chunk-partition
XW = W + KS - 1                            # 132 padded row width
XROWS = ROWS_PER_CORE + KS - 1             # 20 rows incl halo
F32 = mybir.dt.float32

# Reduction: the otherwise-idle TensorEngine sums all 25 tap-product segments
# with identity matmuls accumulating into one PSUM bank (exact f32: 1.0*x is
# exact, PSUM accumulation is f32 add). ScalarE evacuates PSUM -> SBUF. DVE
# does only the products; GpSimd stays idle (no shared-SBUF-port contention).

_compiled = None


def _build_program():
    nc = bacc.Bacc(
        "TRN2",
        target_bir_lowering=False,
        debug=False,
        enable_asserts=False,
        num_devices=N_CORES,
    )
    # Host pre-arranges k as [plane][chunk][tap][h2][w] so each chunk load is
    # one contiguous per-partition run (few DMA descriptors, near line rate).
    xd = nc.declare_dram_parameter("x", [NPLANES, XROWS * XW], F32, isOutput=False)
    kd = nc.declare_dram_parameter(
        "k", [NPLANES, NCHUNK * NTAPS * FDW], F32, isOutput=False
    )
    od = nc.declare_dram_parameter("o", [NPLANES, NCHUNK * FDW], F32, isOutput=True)
    ed = nc.declare_dram_parameter("eye", [NPLANES, NPLANES], F32, isOutput=False)

    with tile.TileContext(nc) as tc:
        with (
            tc.tile_pool(name="xpool", bufs=1) as xpool,
            tc.tile_pool(name="epool", bufs=1) as epool,
            tc.tile_pool(name="kpool", bufs=2) as kpool,
            tc.tile_pool(name="ppool", bufs=2) as ppool,
            tc.tile_pool(name="spool", bufs=3, space="PSUM") as spool,
            tc.tile_pool(name="opool", bufs=3) as opool,
        ):
            # Whole padded input band for this core, resident for the kernel.
            xt = xpool.tile([NPLANES, XROWS * XW], F32)
            nc.sync.dma_start(out=xt[:], in_=xd.ap())
            et = epool.tile([NPLANES, NPLANES], F32)
            nc.sync.dma_start(out=et[:], in_=ed.ap())
            xt_ap = xt[:]
            xt_pdim = xt_ap.ap[0]  # (partition step, 128)

            for ch in range(NCHUNK):
                h0 = ch * ROWS_PER_CHUNK
                kt = kpool.tile([NPLANES, NTAPS * FDW], F32, tag="kt")
                nc.sync.dma_start(
                    out=kt[:],
                    in_=kd.ap()[:, ch * NTAPS * FDW : (ch + 1) * NTAPS * FDW],
                )
                pt = ppool.tile([NPLANES, NTAPS * FDW], F32, tag="pt")
                # Products: one op per vertical tap i covers the 5 horizontal
                # taps j as an overlapping strided window of the X band.
                seg = KS * FDW
                for i in range(KS):
                    k_view = kt[:, i * seg : (i + 1) * seg].rearrange(
                        "p (j h w) -> p j h w", j=KS, h=ROWS_PER_CHUNK, w=W
                    )
                    p_view = pt[:, i * seg : (i + 1) * seg].rearrange(
                        "p (j h w) -> p j h w", j=KS, h=ROWS_PER_CHUNK, w=W
                    )
                    x_view = AP(
                        xt_ap.tensor,
                        xt_ap.offset + (h0 + i) * XW,
                        [xt_pdim, (1, KS), (XW, ROWS_PER_CHUNK), (1, W)],
                    )
                    nc.vector.tensor_mul(p_view, k_view, x_view)

                # TensorE: 25 identity matmuls accumulate all tap segments
                # into one PSUM bank.
                st = spool.tile([NPLANES, FDW], F32, tag="st")
                for t in range(NTAPS):
                    nc.tensor.matmul(
                        st[:],
                        et[:],
                        pt[:, t * FDW : (t + 1) * FDW],
                        start=(t == 0),
                        stop=(t == NTAPS - 1),
                    )

                # ScalarE: evacuate PSUM -> SBUF, then store.
                ot = opool.tile([NPLANES, FDW], F32, tag="ot")
                nc.scalar.copy(ot[:], st[:])
                nc.sync.dma_start(
                    out=od.ap()[:, ch * FDW : (ch + 1) * FDW], in_=ot[:]
                )

    nc.compile()
    return nc


def _get_program():
    global _compiled
    if _compiled is None:
        _compiled = _build_program()
    return _compiled


def _shard_inputs(input: np.ndarray, kernel: np.ndarray):
    x = np.ascontiguousarray(input, dtype=np.float32).reshape(NPLANES, H, W)
    xp = np.pad(x, ((0, 0), (2, 2), (2, 2)), mode="edge")  # [128, 132, 132]
    k = np.ascontiguousarray(kernel, dtype=np.float32).reshape(
        NPLANES, NTAPS, H, W
    )
    eye = np.eye(NPLANES, dtype=np.float32)
    in_maps = []
    for c in range(N_CORES):
        r0 = c * ROWS_PER_CORE
        # [plane][tap][16 rows][w] -> [plane][chunk][tap][2 rows][w], flattened
        # per plane so each chunk is one contiguous run.
        kc = k[:, :, r0 : r0 + ROWS_PER_CORE, :].reshape(
            NPLANES, NTAPS, NCHUNK, ROWS_PER_CHUNK, W
        )
        kc = np.ascontiguousarray(kc.transpose(0, 2, 1, 3, 4)).reshape(
            NPLANES, NCHUNK * NTAPS * FDW
        )
        in_maps.append(
            {
                "x": np.ascontiguousarray(
                    xp[:, r0 : r0 + XROWS, :]
                ).reshape(NPLANES, XROWS * XW),
                "k": kc,
                "eye": eye,
            }
        )
    return in_maps


last_results = None  # BassKernelResults of the most recent run (for profiling)


def kernel(input: np.ndarray, kernel: np.ndarray, _trace: bool = False):
    global last_results
    nc = _get_program()
    in_maps = _shard_inputs(input, kernel)
    res = run_bass_kernel_spmd(nc, in_maps, list(range(N_CORES)), trace=_trace)
    last_results = res
    out = np.empty((NPLANES, H, W), dtype=np.float32)
    for c in range(N_CORES):
        out[:, c * ROWS_PER_CORE : (c + 1) * ROWS_PER_CORE, :] = res.results[c][
            "o"
        ].reshape(NPLANES, ROWS_PER_CORE, W)
    return out.reshape(B, C, H, W)


if __name__ == "__main__":
    rng = np.random.default_rng(0)
    inp = rng.standard_normal((B, C, H, W), dtype=np.float32)
    kern = rng.standard_normal((B, C * NTAPS, H, W), dtype=np.float32)
    out = kernel(inp, kern)
    print("ran ok", out.shape, out.dtype)


# revision 19
# speedup vs baseline: 1.6559x; 1.0346x over previous
"""KernelConv2D (per-pixel dynamic 5x5 depthwise conv) on 8 TRN2 NeuronCores.

Problem: out[b,c,h,w] = sum_{i,j} x_edgepad[b,c,h+i,w+j] * K[b,c,i,j,h,w]
with input [4,32,128,128] f32 and kernel [4,800,128,128] f32 (800 = 32*25).

Sharding: every (b,c) plane is independent, so flatten to 128 planes and put
the plane index on the SBUF partition axis. Each core takes 16 output ROWS of
all 128 planes (row-sharding). With (h, w) both living in the free dimension,
both conv shifts are constant free-dim offsets -> the 5x5 taps of the input
window are expressed as a single overlapping access pattern, no halo exchange
or partition-shifted copies on device. Host pre-pads the input with edge
replication and slices per-core row bands (incl. 2-row halo).

Per core HBM traffic: K 26.2MB + X 1.35MB + out 1.05MB ~= 28.6MB -> ~80us at
~358 GB/s/core: the memory roofline for this problem. Compute is split so DVE
(products + 9-segment reduce) and GpSimd (16-segment add tree) both stay at or
under the DMA time.
"""

import sys

import numpy as np

sys.path.insert(0, "/opt/trn_rl_repo")

import concourse.bacc as bacc
import concourse.bass as bass
import concourse.tile as tile
from concourse import mybir
from concourse.ap import AP
from concourse.bass_utils import run_bass_kernel_spmd

N_CORES = 8
B, C, H, W, KS = 4, 32, 128, 128, 5
NPLANES = B * C          # 128 -> partition axis
NTAPS = KS * KS          # 25
ROWS_PER_CORE = H // N_CORES   # 16
ROWS_PER_CHUNK = 2
NCHUNK = ROWS_PER_CORE // ROWS_PER_CHUNK   # 8
FDW = ROWS_PER_CHUNK * W                   # 256 output elems per chunk-partition
XW = W + KS - 1                            # 132 padded row width
XROWS = ROWS_PER_CORE + KS - 1             # 20 rows incl halo
F32 = mybir.dt.float32

# Reduction: the otherwise-idle TensorEngine sums all 25 tap-product segments
# with identity matmuls accumulating into one PSUM bank (exact f32: 1.0*x is
# exact, PSUM accumulation is f32 add). ScalarE evacuates PSUM -> SBUF. DVE
# does only the products; GpSimd stays idle (no shared-SBUF-port contention).

_compiled = None


def _build_program():
    nc = bacc.Bacc(
        "TRN2",
        target_bir_lowering=False,
        debug=False,
        enable_asserts=False,
        num_devices=N_CORES,
    )
    # Host pre-arranges k as [plane][chunk][tap][h2][w] so each chunk load is
    # one contiguous per-partition run (few DMA descriptors, near line rate).
    xd = nc.declare_dram_parameter("x", [NPLANES, XROWS * XW], F32, isOutput=False)
    kd = nc.declare_dram_parameter(
        "k", [NPLANES, NCHUNK * NTAPS * FDW], F32, isOutput=False
    )
    od = nc.declare_dram_parameter("o", [NPLANES, NCHUNK * FDW], F32, isOutput=True)
    ed = nc.declare_dram_parameter("eye", [NPLANES, NPLANES], F32, isOutput=False)

    with tile.TileContext(nc) as tc:
        with (
            tc.tile_pool(name="xpool", bufs=1) as xpool,
            tc.tile_pool(name="epool", bufs=1) as epool,
            tc.tile_pool(name="kpool", bufs=3) as kpool,
            tc.tile_pool(name="ppool", bufs=2) as ppool,
            tc.tile_pool(name="dpool", bufs=2) as dpool,
            tc.tile_pool(name="spool", bufs=3, space="PSUM") as spool,
            tc.tile_pool(name="opool", bufs=3) as opool,
        ):
            # Whole padded input band for this core, resident for the kernel.
            xt = xpool.tile([NPLANES, XROWS * XW], F32)
            nc.sync.dma_start(out=xt[:], in_=xd.ap())
            et = epool.tile([NPLANES, NPLANES], F32)
            nc.sync.dma_start(out=et[:], in_=ed.ap())
            xt_ap = xt[:]
            xt_pdim = xt_ap.ap[0]  # (partition step, 128)

            for ch in range(NCHUNK):
                h0 = ch * ROWS_PER_CHUNK
                kt = kpool.tile([NPLANES, NTAPS * FDW], F32, tag="kt")
                # Two sub-loads per chunk: products for taps 0-9 only gate on
                # the first half (cuts the startup ramp by ~half a chunk load).
                base = ch * NTAPS * FDW
                nc.sync.dma_start(
                    out=kt[:, 0 : 10 * FDW], in_=kd.ap()[:, base : base + 10 * FDW]
                )
                nc.sync.dma_start(
                    out=kt[:, 10 * FDW :],
                    in_=kd.ap()[:, base + 10 * FDW : base + NTAPS * FDW],
                )
                pt = ppool.tile([NPLANES, NTAPS * FDW], F32, tag="pt")
                # Products: one op per vertical tap i covers the 5 horizontal
                # taps j as an overlapping strided window of the X band.
                seg = KS * FDW
                for i in range(KS):
                    k_view = kt[:, i * seg : (i + 1) * seg].rearrange(
                        "p (j h w) -> p j h w", j=KS, h=ROWS_PER_CHUNK, w=W
                    )
                    p_view = pt[:, i * seg : (i + 1) * seg].rearrange(
                        "p (j h w) -> p j h w", j=KS, h=ROWS_PER_CHUNK, w=W
                    )
                    x_view = AP(
                        xt_ap.tensor,
                        xt_ap.offset + (h0 + i) * XW,
                        [xt_pdim, (1, KS), (XW, ROWS_PER_CHUNK), (1, W)],
                    )
                    nc.vector.tensor_mul(p_view, k_view, x_view)

                # DVE pre-adds 5 tap pairs in one op (taps 0-4 + 5-9) so the
                # 4-pass fp32 PE only accumulates 20 segments, keeping it
                # under the DMA pace.
                dt = dpool.tile([NPLANES, KS * FDW], F32, tag="dt")
                nc.vector.tensor_add(
                    dt[:], pt[:, 0 : KS * FDW], pt[:, KS * FDW : 10 * FDW]
                )

                # TensorE: identity matmuls accumulate the remaining segments
                # into one PSUM bank (exact f32 adds).
                st = spool.tile([NPLANES, FDW], F32, tag="st")
                segs = [pt[:, t * FDW : (t + 1) * FDW] for t in range(10, NTAPS)]
                segs += [dt[:, t * FDW : (t + 1) * FDW] for t in range(KS)]
                for t, s in enumerate(segs):
                    nc.tensor.matmul(
                        st[:], et[:], s, start=(t == 0), stop=(t == len(segs) - 1)
                    )

                # ScalarE: evacuate PSUM -> SBUF, then store.
                ot = opool.tile([NPLANES, FDW], F32, tag="ot")
                nc.scalar.copy(ot[:], st[:])
                nc.sync.dma_start(
                    out=od.ap()[:, ch * FDW : (ch + 1) * FDW], in_=ot[:]
                )

    nc.compile()
    return nc


def _get_program():
    global _compiled
    if _compiled is None:
        _compiled = _build_program()
    return _compiled


def _shard_inputs(input: np.ndarray, kernel: np.ndarray):
    x = np.ascontiguousarray(input, dtype=np.float32).reshape(NPLANES, H, W)
    xp = np.pad(x, ((0, 0), (2, 2), (2, 2)), mode="edge")  # [128, 132, 132]
    k = np.ascontiguousarray(kernel, dtype=np.float32).reshape(
        NPLANES, NTAPS, H, W
    )
    eye = np.eye(NPLANES, dtype=np.float32)
    in_maps = []
    for c in range(N_CORES):
        r0 = c * ROWS_PER_CORE
        # [plane][tap][16 rows][w] -> [plane][chunk][tap][2 rows][w], flattened
        # per plane so each chunk is one contiguous run.
        kc = k[:, :, r0 : r0 + ROWS_PER_CORE, :].reshape(
            NPLANES, NTAPS, NCHUNK, ROWS_PER_CHUNK, W
        )
        kc = np.ascontiguousarray(kc.transpose(0, 2, 1, 3, 4)).reshape(
            NPLANES, NCHUNK * NTAPS * FDW
        )
        in_maps.append(
            {
                "x": np.ascontiguousarray(
                    xp[:, r0 : r0 + XROWS, :]
                ).reshape(NPLANES, XROWS * XW),
                "k": kc,
                "eye": eye,
            }
        )
    return in_maps


last_results = None  # BassKernelResults of the most recent run (for profiling)


def kernel(input: np.ndarray, kernel: np.ndarray, _trace: bool = False):
    global last_results
    nc = _get_program()
    in_maps = _shard_inputs(input, kernel)
    res = run_bass_kernel_spmd(nc, in_maps, list(range(N_CORES)), trace=_trace)
    last_results = res
    out = np.empty((NPLANES, H, W), dtype=np.float32)
    for c in range(N_CORES):
        out[:, c * ROWS_PER_CORE : (c + 1) * ROWS_PER_CORE, :] = res.results[c][
            "o"
        ].reshape(NPLANES, ROWS_PER_CORE, W)
    return out.reshape(B, C, H, W)


if __name__ == "__main__":
    rng = np.random.default_rng(0)
    inp = rng.standard_normal((B, C, H, W), dtype=np.float32)
    kern = rng.standard_normal((B, C * NTAPS, H, W), dtype=np.float32)
    out = kernel(inp, kern)
    print("ran ok", out.shape, out.dtype)
